# revision 5
# baseline (speedup 1.0000x reference)
"""2-layer GATv2 (PyG GATv2Conv semantics) on 8 Trainium2 NeuronCores — v2.

Improvements over v1:
  - bf16 matmul/table/gather datapath (fp32 PSUM accumulation + epilogues).
  - Self-loop edges are NOT gathered: their message/score path is computed
    batched from the resident per-core xl/xr tiles (overlapping the
    AllGather) and injected into the scatter accumulator with one
    identity-stationary matmul per node tile.  This also kills the lo/hi
    table-half imbalance (self edges all hit the own half) and all the
    fake-pad-edge machinery.
  - Host pre-transposes x, so layer-1 projections need no PE transposes;
    W1l|W1r are concatenated into one 512-wide moving operand.
  - One-hot matrices: oh_en (edge-major) via one batched DVE compare per
    half, oh_s (dst-major) via a rank-1 PE broadcast of the host-provided
    transposed slot row + one batched compare.  No per-tile PE transpose,
    no per-tile PSUM->SBUF copies.
  - All per-edge-tile DVE/ACT elementwise ops are batched per (node tile,
    table half): one Prelu, one att-mult, one segmented reduce, one Exp,
    one msg-mult.  Broadcast operands use pair-duplicated layouts to keep
    the DVE in 2x packed mode.
  - Layer 2: ex folded into the one-hot (ohx), denominator via a
    1.0-column in the gather table rows, log_softmax batched across all
    node tiles.
"""

import sys
import os

if "/opt/trn_rl_repo" not in sys.path:
    sys.path.insert(0, "/opt/trn_rl_repo")

import numpy as np
import ml_dtypes

NC = 8          # cores
P = 128         # partitions
NEG_SLOPE = 0.2

_plan_cache = {}


# --------------------------------------------------------------------------
# host-side graph preprocessing
# --------------------------------------------------------------------------

def _snake(order, nbins):
    n = len(order)
    ids = np.arange(n)
    round_ = ids // nbins
    pos = ids % nbins
    b = np.where(round_ % 2 == 0, pos, nbins - 1 - pos)
    out = np.empty(n, np.int64)
    out[:] = b
    return out


def _preprocess(N, E, edge_index):
    NPC = ((N + NC - 1) // NC + P - 1) // P * P    # padded nodes per core
    NT = NPC // P
    TBL = NC * NPC
    assert TBL // 2 < 32768, "table half must fit int16 row indices"

    src = edge_index[0].astype(np.int64)           # real edges only
    dst = edge_index[1].astype(np.int64)
    deg = np.bincount(dst, minlength=N)

    # --- core assignment: snake over degree-sorted nodes
    order = np.argsort(-deg, kind="stable")
    core_of = np.empty(N, np.int64)
    core_of[order] = _snake(order, NC)

    lo_src = core_of[src] < NC // 2                # which table half each edge reads
    deg_lo = np.bincount(dst[lo_src], minlength=N)
    deg_hi = deg - deg_lo

    # --- per-core tile packing: greedy 2D balance of (lo, hi) in-edges
    local_of = np.empty(N, np.int64)
    for c in range(NC):
        nodes = np.where(core_of == c)[0]
        nodes = nodes[np.argsort(-deg[nodes], kind="stable")]
        Lt = np.zeros(NT)
        Ht = np.zeros(NT)
        cnt = np.zeros(NT, np.int64)
        lo_v = deg_lo[nodes].astype(np.float64)
        hi_v = deg_hi[nodes].astype(np.float64)
        tile_ids = np.empty(len(nodes), np.int64)
        for i in range(len(nodes)):
            cost = np.maximum(Lt + lo_v[i], Ht + hi_v[i])
            cost[cnt >= P] = np.inf
            tl = int(np.argmin(cost))
            tile_ids[i] = tl
            Lt[tl] += lo_v[i]
            Ht[tl] += hi_v[i]
            cnt[tl] += 1
        slot = np.zeros(NT, np.int64)
        for v, tl in zip(nodes, tile_ids):
            local_of[v] = tl * P + slot[tl]
            slot[tl] += 1

    r_of = core_of * NPC + local_of                # global table row of each node

    # --- per (core, tile) edge lists split by half
    e_core = core_of[dst]
    e_tile = local_of[dst] // P
    e_slot = local_of[dst] % P
    cnt_lo = np.zeros((NC, NT), np.int64)
    cnt_hi = np.zeros((NC, NT), np.int64)
    np.add.at(cnt_lo, (e_core[lo_src], e_tile[lo_src]), 1)
    np.add.at(cnt_hi, (e_core[~lo_src], e_tile[~lo_src]), 1)
    K_lo = int(max(1, ((cnt_lo + P - 1) // P).max()))
    K_hi = int(max(1, ((cnt_hi + P - 1) // P).max()))
    T = K_lo + K_hi

    def pack_idx(flat):
        n = len(flat)
        s = (n + 15) // 16
        arr = np.zeros(s * 16, np.int16)
        arr[:n] = flat
        block = arr.reshape(s, 16).T
        return np.tile(block, (8, 1))

    gidx = np.zeros((NC, P, NT * T * 8), np.int16)
    drel = np.full((NC, P, NT * T), -1.0, np.float32)
    for c in range(NC):
        m_c = e_core == c
        for tl in range(NT):
            m = m_c & (e_tile == tl)
            ml = m & lo_src
            mh = m & ~lo_src
            for half, (K, rows0, slots0) in enumerate(
                    [(K_lo, r_of[src[ml]], e_slot[ml]),
                     (K_hi, r_of[src[mh]] - TBL // 2, e_slot[mh])]):
                n = len(rows0)
                flat = np.zeros(K * P, np.int64)
                flat[:n] = rows0
                off = (tl * T + (K_lo if half else 0)) * 8
                gidx[c, :, off:off + K * 8] = pack_idx(flat)
                dr = np.full(K * P, -1.0, np.float32)
                dr[:n] = slots0
                tcol0 = tl * T + (K_lo if half else 0)
                drel[c, :, tcol0:tcol0 + K] = dr.reshape(K, P).T

    # pair-duplicated drel (for the oh_en compare broadcast) and the
    # transposed flat slot row (for the rank-1 oh_s broadcast)
    drelD = np.repeat(drel, 2, axis=2).reshape(NC, P, NT * T, 2)
    drelT = np.ascontiguousarray(
        np.transpose(drel, (0, 2, 1)).reshape(NC, 1, NT * T * P))

    node_order = np.full((NC, NPC), -1, np.int64)  # local row -> global node id
    for c in range(NC):
        nodes = np.where(core_of == c)[0]
        node_order[c, local_of[nodes]] = nodes

    return dict(NPC=NPC, NT=NT, TBL=TBL, K_lo=K_lo, K_hi=K_hi, T=T,
                gidx=gidx, drel=drel, drelD=drelD, drelT=drelT,
                node_order=node_order, core_of=core_of, local_of=local_of)


# --------------------------------------------------------------------------
# bass program
# --------------------------------------------------------------------------

def _build_program(dims, post_passes=True):
    import concourse.bass as bass
    import concourse.mybir as mybir
    import concourse.tile as tile
    from concourse import library_config
    from concourse.bass import _add_dep_helper
    import bass_rust as _br

    fp32 = mybir.dt.float32
    bf = mybir.dt.bfloat16
    i16 = mybir.dt.int16
    AX = mybir.AxisListType
    OP = mybir.AluOpType
    AF = mybir.ActivationFunctionType

    DIN = dims["DIN"]; HC = dims["HC"]; H = dims["H"]; CH = dims["CH"]
    CO = dims["CO"]
    NPC = dims["NPC"]; NT = dims["NT"]; TBL = dims["TBL"]
    K_lo = dims["K_lo"]; K_hi = dims["K_hi"]; T = dims["T"]
    KD = DIN // P
    KH = HC // P
    CO_PAD = 128                       # 256B gather rows in bf16
    HALF = TBL // 2
    OHS_ACT = os.environ.get("GAT_OHS_ACT", "1") == "1"
    SHARED = os.environ.get("GAT_SHARED", "0") == "1"

    nc = bass.Bass(num_devices=NC)

    xkT = nc.dram_tensor("xkT", [P, NT * KD * P], bf, kind="ExternalInput")
    w1lr = nc.dram_tensor("w1lr", [P, KD * 2 * HC], bf, kind="ExternalInput")
    w2lr = nc.dram_tensor("w2lr", [P, KH * 2 * CO], bf, kind="ExternalInput")
    # consts: iota(P) | ident(P) | attB(HC) | att2B(CO)
    CC = P + P + HC + CO
    consts = nc.dram_tensor("consts", [P, CC], bf, kind="ExternalInput")
    constf = nc.dram_tensor("constf", [P, 2], fp32, kind="ExternalInput")  # alpha|colidx
    ones1 = nc.dram_tensor("ones1", [1, P], bf, kind="ExternalInput")
    gidx_d = nc.dram_tensor("gidx", [P, NT * T * 8], i16, kind="ExternalInput")
    drelD_d = nc.dram_tensor("drelD", [P, NT * T * 2], bf, kind="ExternalInput")
    drelT_d = nc.dram_tensor("drelT", [1, NT * T * P], bf, kind="ExternalInput")
    h2_out = nc.dram_tensor("h2o", [NPC, CO], fp32, kind="ExternalOutput")
    ls_out = nc.dram_tensor("lso", [NPC, CO], fp32, kind="ExternalOutput")
    DEBUG = os.environ.get("GAT_DEBUG", "0") == "1"
    if DEBUG:
        xl1_o = nc.dram_tensor("xl1o", [NPC, HC], bf, kind="ExternalOutput")
        xr1_o = nc.dram_tensor("xr1o", [NPC, HC], bf, kind="ExternalOutput")
        m1_o = nc.dram_tensor("m1o", [NPC, HC + H], bf, kind="ExternalOutput")
        h1_o = nc.dram_tensor("h1o", [NPC, HC], bf, kind="ExternalOutput")
        x2_o = nc.dram_tensor("x2o", [NPC, 2 * CO], bf, kind="ExternalOutput")
        glo_o = nc.dram_tensor("gloo", [P, K_lo * HC], bf, kind="ExternalOutput")
        ohe_o = nc.dram_tensor("oheo", [P, K_lo * P], bf, kind="ExternalOutput")
        ohs_o = nc.dram_tensor("ohso", [P, K_lo * P], bf, kind="ExternalOutput")
        md_o = nc.dram_tensor("mdo", [P, K_lo * (HC + H)], bf,
                              kind="ExternalOutput")
        acc_o = nc.dram_tensor("acco", [P, HC + H], fp32, kind="ExternalOutput")

    shr = dict(addr_space="Shared") if SHARED else {}

    with tile.TileContext(nc) as tc:
        with (
            tc.tile_pool(name="dram", bufs=1, space="DRAM") as dram,
            tc.tile_pool(name="cst", bufs=1) as cst,
        ):
            lib = nc.gpsimd.load_library(library_config.mlp)
            reg_klo = nc.gpsimd.to_reg(K_lo * P)
            reg_khi = nc.gpsimd.to_reg(K_hi * P)

            ctile = cst.tile([P, CC], bf)
            nc.sync.dma_start(out=ctile[:], in_=consts[:])
            cftile = cst.tile([P, 2], fp32)
            nc.sync.dma_start(out=cftile[:], in_=constf[:])
            iota = ctile[:, 0:P]
            ident = ctile[:, P:2 * P]
            attB = ctile[:, 2 * P:2 * P + HC]
            att2B = ctile[:, 2 * P + HC:2 * P + HC + CO]
            alpha = cftile[:, 0:1]
            colidx = cftile[:, 1:2]

            ones_sb = cst.tile([1, P], bf)
            nc.sync.dma_start(out=ones_sb[:], in_=ones1[:])

            w1lr_sb = cst.tile([P, KD, 2 * HC], bf)
            nc.sync.dma_start(out=w1lr_sb[:],
                              in_=w1lr.rearrange("p (k c) -> p k c", k=KD))
            w2lr_sb = cst.tile([P, KH, 2 * CO], bf)
            nc.sync.dma_start(out=w2lr_sb[:],
                              in_=w2lr.rearrange("p (k c) -> p k c", k=KH))

            gidx_sb = cst.tile([P, NT * T * 8], i16)
            nc.sync.dma_start(out=gidx_sb[:], in_=gidx_d[:])
            drelD_sb = cst.tile([P, NT * T, 2], bf)
            nc.sync.dma_start(out=drelD_sb[:],
                              in_=drelD_d.rearrange("p (t two) -> p t two", two=2))

            xl1_all = cst.tile([P, NT, HC], bf)
            xr1_all = cst.tile([P, NT, HC], bf)
            msg1s = cst.tile([P, NT, HC + H], bf)
            h1_all = cst.tile([P, NT, HC], bf)
            xl2_all = cst.tile([P, NT, CO], bf)
            xr2_all = cst.tile([P, NT, CO], bf)
            msg2s = cst.tile([P, NT, CO + 1], bf)
            h2_all = cst.tile([P, NT, CO], fp32)
            ls_all = cst.tile([P, NT, CO], fp32)

            ag1_in = dram.tile([NPC, HC], bf)
            tbl1 = dram.tile([TBL, HC], bf, **shr)
            ag2_in = dram.tile([NPC, CO_PAD], bf)
            tbl2 = dram.tile([TBL, CO_PAD], bf, **shr)

            # ============ phase A: layer-1 projections ============
            with (tc.tile_pool(name="sbA", bufs=2) as sb,
                  tc.tile_pool(name="psA", bufs=2, space="PSUM") as ps):
                for nt in range(NT):
                    xT = sb.tile([P, KD, P], bf, tag="xT")
                    nc.sync.dma_start(
                        out=xT[:],
                        in_=xkT.rearrange("p (n k j) -> p n k j", n=NT, k=KD)[:, nt])
                    lr_ps = ps.tile([P, 2 * HC], fp32, tag="mmA", space="PSUM")
                    for k in range(KD):
                        nc.tensor.matmul(out=lr_ps[:], lhsT=xT[:, k, :],
                                         rhs=w1lr_sb[:, k, :],
                                         start=(k == 0), stop=(k == KD - 1))
                    nc.vector.tensor_copy(out=xl1_all[:, nt, :], in_=lr_ps[:, 0:HC])
                    nc.vector.tensor_copy(out=xr1_all[:, nt, :],
                                          in_=lr_ps[:, HC:2 * HC])
                    nc.sync.dma_start(out=ag1_in[nt * P:(nt + 1) * P, :],
                                      in_=xl1_all[:, nt, :])

            nc.gpsimd.collective_compute(
                "AllGather", mybir.AluOpType.bypass,
                replica_groups=[list(range(NC))],
                ins=[ag1_in[:].opt()], outs=[tbl1[:].opt()],
            )

            # ---- self-edge pass (overlaps AG1: no dependence on tbl1) ----
            with tc.tile_pool(name="sbS", bufs=2) as sb:
                CH8 = 8
                for c0 in range(0, NT, CH8):
                    n = min(CH8, NT - c0)
                    sl = slice(c0, c0 + n)
                    zs = sb.tile([P, CH8, HC], bf, tag="zs")
                    nc.vector.tensor_tensor(out=zs[:, 0:n], in0=xl1_all[:, sl],
                                            in1=xr1_all[:, sl], op=OP.add)
                    ts = sb.tile([P, CH8, HC], bf, tag="ts")
                    nc.scalar.activation(out=ts[:, 0:n], in_=zs[:, 0:n],
                                         func=AF.Prelu, alpha=alpha)
                    nc.vector.tensor_tensor(
                        out=ts[:, 0:n], in0=ts[:, 0:n],
                        in1=attB[:, None, :].to_broadcast([P, n, HC]), op=OP.mult)
                    scs = sb.tile([P, CH8, H], fp32, tag="scs")
                    nc.vector.tensor_reduce(
                        out=scs[:, 0:n],
                        in_=ts[:, 0:n].rearrange("p n (h c) -> p n h c", h=H),
                        axis=AX.X, op=OP.add)
                    nc.scalar.activation(out=msg1s[:, sl, HC:HC + H],
                                         in_=scs[:, 0:n], func=AF.Exp)
                    exd = sb.tile([P, CH8, H, 2], bf, tag="exd")
                    nc.vector.tensor_copy(
                        out=exd[:, 0:n],
                        in_=msg1s[:, sl, HC:HC + H][:, :, :, None]
                        .to_broadcast([P, n, H, 2]))
                    nc.vector.tensor_tensor(
                        out=msg1s[:, sl, 0:HC].rearrange(
                            "p n (h c two) -> p n h c two", h=H, two=2),
                        in0=xl1_all[:, sl].rearrange(
                            "p n (h c two) -> p n h c two", h=H, two=2),
                        in1=exd[:, 0:n, :, None, :].to_broadcast(
                            [P, n, H, CH // 2, 2]),
                        op=OP.mult)

            # ============ phase B: layer-1 edges ============
            with (tc.tile_pool(name="sbB", bufs=2) as sb,
                  tc.tile_pool(name="psB", bufs=1, space="PSUM") as ps,
                  tc.tile_pool(name="psBa", bufs=1, space="PSUM") as psa):
                for nt in range(NT):
                    drelT_sb = sb.tile([1, T * P], bf, tag="drelT")
                    nc.sync.dma_start(out=drelT_sb[:],
                                      in_=drelT_d[:, nt * T * P:(nt + 1) * T * P])
                    glo = sb.tile([P, K_lo, HC], bf, tag="glo")
                    ghi = sb.tile([P, K_hi, HC], bf, tag="ghi")
                    off = nt * T * 8
                    g1 = nc.gpsimd.dma_gather(
                        glo[:], tbl1[0:HALF, :], gidx_sb[:, off:off + K_lo * 8],
                        K_lo * P, reg_klo, HC)
                    g2 = nc.gpsimd.dma_gather(
                        ghi[:], tbl1[HALF:TBL, :],
                        gidx_sb[:, off + K_lo * 8:off + T * 8],
                        K_hi * P, reg_khi, HC)
                    _add_dep_helper(g1.ins, lib.ins, sync=False, reason="lib")
                    _add_dep_helper(g2.ins, lib.ins, sync=False, reason="lib")

                    acc = psa.tile([P, HC + H], fp32, tag="acc", space="PSUM")
                    nc.tensor.matmul(out=acc[:], lhsT=ident, rhs=msg1s[:, nt, :],
                                     start=True, stop=False)

                    for half in range(2):
                        K = K_lo if half == 0 else K_hi
                        g = glo if half == 0 else ghi
                        t0c = nt * T + (K_lo if half else 0)
                        doff = (K_lo if half else 0) * P
                        z = ps.tile([P, K, HC], fp32, tag="z", space="PSUM")
                        zf = z[:].rearrange("p k c -> p (k c)")
                        ohe = sb.tile([P, K, P], bf, tag=f"ohe{half}")
                        nc.vector.tensor_tensor(
                            out=ohe[:].rearrange("p k (q two) -> p k q two", two=2),
                            in0=iota[:, None, :].to_broadcast([P, K, P]).rearrange(
                                "p k (q two) -> p k q two", two=2),
                            in1=drelD_sb[:, t0c:t0c + K, None, :].to_broadcast(
                                [P, K, P // 2, 2]),
                            op=OP.is_equal)
                        # oh_s = oh_en^T via PE transpose (dedicated PSUM area)
                        ohsT = ps.tile([P, K_lo, P], fp32, tag="ohsT",
                                       space="PSUM")
                        for t in range(K):
                            nc.tensor.matmul(out=ohsT[:, t, :],
                                             lhsT=ohe[:, t, :], rhs=ident,
                                             start=True, stop=True)
                        ohs = sb.tile([P, K, P], bf, tag=f"ohs{half}")
                        nc.scalar.copy(
                            out=ohs[:].rearrange("p k q -> p (k q)"),
                            in_=ohsT[:].rearrange("p k q -> p (k q)")[:, 0:K * P])
                        # z = xr[dst] + xl[src], per-tile paired accumulation
                        for t in range(K):
                            nc.tensor.matmul(out=z[:, t, :], lhsT=ohs[:, t, :],
                                             rhs=xr1_all[:, nt, :],
                                             start=True, stop=False)
                            nc.tensor.matmul(out=z[:, t, :], lhsT=ident,
                                             rhs=g[:, t, :],
                                             start=False, stop=True)
                        tb = sb.tile([P, K, HC], bf, tag=f"tb{half}")
                        nc.scalar.activation(
                            out=tb[:].rearrange("p k c -> p (k c)"),
                            in_=zf[:], func=AF.Prelu, alpha=alpha)
                        nc.vector.tensor_tensor(
                            out=tb[:], in0=tb[:],
                            in1=attB[:, None, :].to_broadcast([P, K, HC]),
                            op=OP.mult)
                        scf = sb.tile([P, K, H], fp32, tag=f"sc{half}")
                        nc.vector.tensor_reduce(
                            out=scf[:],
                            in_=tb[:].rearrange("p k (h c) -> p k h c", h=H),
                            axis=AX.X, op=OP.add)
                        md = sb.tile([P, K, HC + H], bf, tag=f"md{half}")
                        nc.scalar.activation(out=md[:, :, HC:HC + H], in_=scf[:],
                                             func=AF.Exp)
                        exd = sb.tile([P, K, H, 2], bf, tag=f"exd{half}")
                        nc.vector.tensor_copy(
                            out=exd[:],
                            in_=md[:, :, HC:HC + H][:, :, :, None]
                            .to_broadcast([P, K, H, 2]))
                        nc.vector.tensor_tensor(
                            out=md[:, :, 0:HC].rearrange(
                                "p k (h c two) -> p k h c two", h=H, two=2),
                            in0=g[:].rearrange(
                                "p k (h c two) -> p k h c two", h=H, two=2),
                            in1=exd[:, :, :, None, :].to_broadcast(
                                [P, K, H, CH // 2, 2]),
                            op=OP.mult)
                        for t in range(K):
                            nc.tensor.matmul(
                                out=acc[:], lhsT=ohe[:, t, :], rhs=md[:, t, :],
                                start=False, stop=(half == 1 and t == K - 1))
                        if DEBUG and nt == 0 and half == 0:
                            nc.sync.dma_start(
                                out=glo_o[:],
                                in_=g[:].rearrange("p k c -> p (k c)"))
                            nc.sync.dma_start(
                                out=ohe_o[:],
                                in_=ohe[:].rearrange("p k q -> p (k q)"))
                            nc.sync.dma_start(
                                out=ohs_o[:],
                                in_=ohs[:].rearrange("p k q -> p (k q)"))
                            nc.sync.dma_start(
                                out=md_o[:],
                                in_=md[:].rearrange("p k c -> p (k c)"))

                    if DEBUG and nt == 0:
                        accc = sb.tile([P, HC + H], fp32, tag="accc")
                        nc.vector.tensor_copy(out=accc[:], in_=acc[:])
                        nc.sync.dma_start(out=acc_o[:], in_=accc[:])

                    # ---- per-node epilogue: softmax divide, elu, layer-2 proj
                    rec = sb.tile([P, H], fp32, tag="rec")
                    nc.vector.reciprocal(out=rec[:], in_=acc[:, HC:HC + H])
                    recd = sb.tile([P, H, 2], fp32, tag="recd")
                    nc.vector.tensor_copy(
                        out=recd[:], in_=rec[:, :, None].to_broadcast([P, H, 2]))
                    nc.vector.tensor_tensor(
                        out=h1_all[:, nt, :].rearrange(
                            "p (h c two) -> p h c two", h=H, two=2),
                        in0=acc[:, 0:HC].rearrange(
                            "p (h c two) -> p h c two", h=H, two=2),
                        in1=recd[:, :, None, :].to_broadcast([P, H, CH // 2, 2]),
                        op=OP.mult)
                    h1v = h1_all[:, nt, :]
                    ehb = sb.tile([P, HC], bf, tag="ehb")
                    nc.scalar.activation(out=ehb[:], in_=h1v, func=AF.Exp)
                    em = sb.tile([P, HC], bf, tag="em")
                    nc.vector.tensor_scalar(out=em[:], in0=ehb[:], scalar1=1.0,
                                            scalar2=0.0, op0=OP.subtract,
                                            op1=OP.min)
                    elu = sb.tile([P, HC], bf, tag="elu")
                    nc.vector.tensor_scalar(out=elu[:], in0=h1v, scalar1=0.0,
                                            scalar2=None, op0=OP.max)
                    nc.vector.tensor_tensor(out=elu[:], in0=elu[:], in1=em[:],
                                            op=OP.add)
                    hT_ps = ps.tile([P, KH, P], fp32, tag="hT", space="PSUM")
                    for k in range(KH):
                        nc.tensor.matmul(out=hT_ps[:, k, :],
                                         lhsT=elu[:, k * P:(k + 1) * P],
                                         rhs=ident, start=True, stop=True)
                    hT_sb = sb.tile([P, KH, P], bf, tag="hTs")
                    nc.vector.tensor_copy(out=hT_sb[:], in_=hT_ps[:])
                    x2_ps = ps.tile([P, 2 * CO], fp32, tag="x2", space="PSUM")
                    for k in range(KH):
                        nc.tensor.matmul(out=x2_ps[:], lhsT=hT_sb[:, k, :],
                                         rhs=w2lr_sb[:, k, :],
                                         start=(k == 0), stop=(k == KH - 1))
                    nc.vector.tensor_copy(out=xl2_all[:, nt, :],
                                          in_=x2_ps[:, 0:CO])
                    nc.vector.tensor_copy(out=xr2_all[:, nt, :],
                                          in_=x2_ps[:, CO:2 * CO])
                    x2c = sb.tile([P, CO + 1], bf, tag="x2c")
                    nc.vector.tensor_copy(out=x2c[:, 0:CO], in_=xl2_all[:, nt, :])
                    nc.vector.memset(x2c[:, CO:CO + 1], 1.0)
                    nc.sync.dma_start(
                        out=ag2_in[nt * P:(nt + 1) * P, 0:CO + 1], in_=x2c[:])

            nc.gpsimd.collective_compute(
                "AllGather", mybir.AluOpType.bypass,
                replica_groups=[list(range(NC))],
                ins=[ag2_in[:].opt()], outs=[tbl2[:].opt()],
            )

            # ---- layer-2 self-edge pass (overlaps AG2) ----
            with tc.tile_pool(name="sbS2", bufs=1) as sb:
                z2s = sb.tile([P, NT, CO], bf)
                nc.vector.tensor_tensor(out=z2s[:], in0=xl2_all[:],
                                        in1=xr2_all[:], op=OP.add)
                t2s = sb.tile([P, NT, CO], bf)
                nc.scalar.activation(out=t2s[:], in_=z2s[:], func=AF.Prelu,
                                     alpha=alpha)
                nc.vector.tensor_tensor(
                    out=t2s[:], in0=t2s[:],
                    in1=att2B[:, None, :].to_broadcast([P, NT, CO]), op=OP.mult)
                sc2s = sb.tile([P, NT], fp32)
                nc.vector.tensor_reduce(out=sc2s[:], in_=t2s[:], axis=AX.X,
                                        op=OP.add)
                ex2s = sb.tile([P, NT], bf)
                nc.scalar.activation(out=ex2s[:], in_=sc2s[:], func=AF.Exp)
                nc.vector.tensor_copy(out=msg2s[:, :, CO:CO + 1],
                                      in_=ex2s[:, :, None])
                ex2sd = sb.tile([P, NT, 2], bf)
                nc.vector.tensor_copy(
                    out=ex2sd[:], in_=ex2s[:, :, None].to_broadcast([P, NT, 2]))
                nc.vector.tensor_tensor(
                    out=msg2s[:, :, 0:CO].rearrange(
                        "p n (c two) -> p n c two", two=2),
                    in0=xl2_all[:].rearrange("p n (c two) -> p n c two", two=2),
                    in1=ex2sd[:, :, None, :].to_broadcast([P, NT, CO // 2, 2]),
                    op=OP.mult)

            # ============ phase C: layer-2 edges ============
            with (tc.tile_pool(name="sbC", bufs=2) as sb,
                  tc.tile_pool(name="psC", bufs=1, space="PSUM") as ps,
                  tc.tile_pool(name="psCa", bufs=2, space="PSUM") as psa):
                for nt in range(NT):
                    drelT_sb = sb.tile([1, T * P], bf, tag="drelT")
                    nc.sync.dma_start(out=drelT_sb[:],
                                      in_=drelT_d[:, nt * T * P:(nt + 1) * T * P])
                    g2lo = sb.tile([P, K_lo, CO_PAD], bf, tag="g2lo")
                    g2hi = sb.tile([P, K_hi, CO_PAD], bf, tag="g2hi")
                    off = nt * T * 8
                    g1 = nc.gpsimd.dma_gather(
                        g2lo[:], tbl2[0:HALF, :], gidx_sb[:, off:off + K_lo * 8],
                        K_lo * P, reg_klo, CO_PAD)
                    g2 = nc.gpsimd.dma_gather(
                        g2hi[:], tbl2[HALF:TBL, :],
                        gidx_sb[:, off + K_lo * 8:off + T * 8],
                        K_hi * P, reg_khi, CO_PAD)
                    _add_dep_helper(g1.ins, lib.ins, sync=False, reason="lib")
                    _add_dep_helper(g2.ins, lib.ins, sync=False, reason="lib")

                    acc2 = psa.tile([P, CO + 1], fp32, tag="acc2", space="PSUM")
                    nc.tensor.matmul(out=acc2[:], lhsT=ident, rhs=msg2s[:, nt, :],
                                     start=True, stop=False)

                    z2 = ps.tile([P, T, CO], fp32, tag="z2", space="PSUM")
                    z2b = sb.tile([P, T, CO], bf, tag="z2b")
                    t2 = sb.tile([P, T, CO], bf, tag="t2")
                    sc2 = sb.tile([P, T], fp32, tag="sc2")
                    ex2 = sb.tile([P, T], bf, tag="ex2")
                    exd2 = sb.tile([P, T, 2], bf, tag="exd2")
                    ohs_l = []
                    ohe_l = []
                    for half in range(2):
                        K = K_lo if half == 0 else K_hi
                        tg = 0 if half == 0 else K_lo
                        t0c = nt * T + tg
                        doff = tg * P
                        dT = ps.tile([P, K_lo * P], fp32, tag=f"dT{half}",
                                     space="PSUM")
                        ohe = sb.tile([P, K, P], bf, tag=f"ohe{half}")
                        nc.vector.tensor_tensor(
                            out=ohe[:].rearrange("p k (q two) -> p k q two", two=2),
                            in0=iota[:, None, :].to_broadcast([P, K, P]).rearrange(
                                "p k (q two) -> p k q two", two=2),
                            in1=drelD_sb[:, t0c:t0c + K, None, :].to_broadcast(
                                [P, K, P // 2, 2]),
                            op=OP.is_equal)
                        dTv = dT[:, 0:K * P].rearrange("p (k q) -> p k q", k=K)
                        for t in range(K):
                            nc.tensor.matmul(out=dTv[:, t, :],
                                             lhsT=ohe[:, t, :], rhs=ident,
                                             start=True, stop=True)
                        ohs = sb.tile([P, K, P], bf, tag=f"ohs{half}")
                        nc.scalar.copy(out=ohs[:].rearrange("p k q -> p (k q)"),
                                       in_=dT[:, 0:K * P])
                        ohs_l.append(ohs)
                        ohe_l.append(ohe)
                        for t in range(K):
                            nc.tensor.matmul(out=z2[:, tg + t, :],
                                             lhsT=ohs[:, t, :],
                                             rhs=xr2_all[:, nt, :],
                                             start=True, stop=True)
                        g = g2lo if half == 0 else g2hi
                        nc.vector.tensor_tensor(
                            out=z2b[:, tg:tg + K, :], in0=z2[:, tg:tg + K, :],
                            in1=g[:, :, 0:CO], op=OP.add)
                    nc.scalar.activation(
                        out=t2[:].rearrange("p t c -> p (t c)"),
                        in_=z2b[:].rearrange("p t c -> p (t c)"),
                        func=AF.Prelu, alpha=alpha)
                    nc.vector.tensor_tensor(
                        out=t2[:], in0=t2[:],
                        in1=att2B[:, None, :].to_broadcast([P, T, CO]),
                        op=OP.mult)
                    nc.vector.tensor_reduce(out=sc2[:], in_=t2[:], axis=AX.X,
                                            op=OP.add)
                    nc.scalar.activation(out=ex2[:], in_=sc2[:], func=AF.Exp)
                    nc.vector.tensor_copy(
                        out=exd2[:], in_=ex2[:, :, None].to_broadcast([P, T, 2]))
                    for half in range(2):
                        K = K_lo if half == 0 else K_hi
                        tg = 0 if half == 0 else K_lo
                        g = g2lo if half == 0 else g2hi
                        ohx = sb.tile([P, K, P], bf, tag=f"ohx{half}")
                        nc.vector.tensor_tensor(
                            out=ohx[:].rearrange("p k (q two) -> p k q two", two=2),
                            in0=ohe_l[half][:].rearrange(
                                "p k (q two) -> p k q two", two=2),
                            in1=exd2[:, tg:tg + K, None, :].to_broadcast(
                                [P, K, P // 2, 2]),
                            op=OP.mult)
                        for t in range(K):
                            nc.tensor.matmul(
                                out=acc2[:], lhsT=ohx[:, t, :],
                                rhs=g[:, t, 0:CO + 1],
                                start=False, stop=(half == 1 and t == K - 1))
                    rec2 = sb.tile([P, 1], fp32, tag="rec2")
                    nc.vector.reciprocal(out=rec2[:], in_=acc2[:, CO:CO + 1])
                    nc.vector.tensor_scalar(out=h2_all[:, nt, :],
                                            in0=acc2[:, 0:CO],
                                            scalar1=rec2[:, 0:1], scalar2=None,
                                            op0=OP.mult)

            # ============ phase D: batched log_softmax ============
            with tc.tile_pool(name="sbD", bufs=1) as sb:
                nm = sb.tile([P, NT], fp32)
                nc.vector.tensor_reduce(out=nm[:], in_=h2_all[:], axis=AX.X,
                                        op=OP.max, negate=True)
                hs = sb.tile([P, NT, CO], fp32)
                nc.vector.tensor_tensor(
                    out=hs[:], in0=h2_all[:],
                    in1=nm[:, :, None].to_broadcast([P, NT, CO]), op=OP.add)
                es = sb.tile([P, NT, CO], fp32)
                nc.scalar.activation(out=es[:].rearrange("p t c -> p (t c)"),
                                     in_=hs[:].rearrange("p t c -> p (t c)"),
                                     func=AF.Exp)
                ssum = sb.tile([P, NT], fp32)
                nc.vector.tensor_reduce(out=ssum[:], in_=es[:], axis=AX.X,
                                        op=OP.add)
                lns = sb.tile([P, NT], fp32)
                nc.scalar.activation(out=lns[:], in_=ssum[:], func=AF.Ln)
                nc.vector.tensor_tensor(
                    out=ls_all[:], in0=hs[:],
                    in1=lns[:, :, None].to_broadcast([P, NT, CO]),
                    op=OP.subtract)

            nc.sync.dma_start(out=h2_out.rearrange("(a p) d -> p a d", p=P),
                              in_=h2_all[:])
            nc.sync.dma_start(out=ls_out.rearrange("(a p) d -> p a d", p=P),
                              in_=ls_all[:])
            if DEBUG:
                for name_t, src_t in [(xl1_o, xl1_all), (xr1_o, xr1_all),
                                      (m1_o, msg1s), (h1_o, h1_all)]:
                    nc.sync.dma_start(
                        out=name_t.rearrange("(a p) d -> p a d", p=P),
                        in_=src_t[:])
                nc.sync.dma_start(
                    out=x2_o.rearrange("(a p) (d) -> p a d", p=P)[:, :, 0:CO],
                    in_=xl2_all[:])
                nc.sync.dma_start(
                    out=x2_o.rearrange("(a p) (d) -> p a d", p=P)[:, :, CO:2 * CO],
                    in_=xr2_all[:])

    if post_passes:
        _br.generate_event_semaphores(nc)
        _br.codegen_inst_isa_subclasses(nc)
    return nc


# --------------------------------------------------------------------------
# entry point
# --------------------------------------------------------------------------

def kernel(x, edge_index, W1l, W1r, att1, b1, W2l, W2r, att2, b2):
    x = np.asarray(x, np.float32)
    edge_index = np.asarray(edge_index)
    W1l = np.asarray(W1l, np.float32); W1r = np.asarray(W1r, np.float32)
    att1 = np.asarray(att1, np.float32); b1 = np.asarray(b1, np.float32)
    W2l = np.asarray(W2l, np.float32); W2r = np.asarray(W2r, np.float32)
    att2 = np.asarray(att2, np.float32); b2 = np.asarray(b2, np.float32)

    N, DIN = x.shape
    E = edge_index.shape[1]
    H, CH = att1.shape
    HC = W1l.shape[1]
    CO = W2l.shape[1]
    assert np.abs(b1).max() == 0 and np.abs(b2).max() == 0, "bias not supported"

    key = (N, E, DIN, H, CH, HC, CO, hash(edge_index.tobytes()))
    if key in _plan_cache:
        pp, nc, dims = _plan_cache[key]
    else:
        pp = _preprocess(N, E, edge_index)
        dims = dict(DIN=DIN, HC=HC, H=H, CH=CH, CO=CO,
                    NPC=pp["NPC"], NT=pp["NT"], TBL=pp["TBL"],
                    K_lo=pp["K_lo"], K_hi=pp["K_hi"], T=pp["T"])
        nc = _build_program(dims)
        _plan_cache[key] = (pp, nc, dims)

    NPC = pp["NPC"]; NT = pp["NT"]
    KD = DIN // P
    KH = HC // P
    bfdt = ml_dtypes.bfloat16

    iota = np.broadcast_to(np.arange(P, dtype=np.float32)[None, :], (P, P))
    ident = np.eye(P, dtype=np.float32)
    attB = np.broadcast_to(att1.reshape(1, HC), (P, HC))
    att2B = np.broadcast_to(att2.reshape(1, CO), (P, CO))
    consts = np.concatenate([iota, ident, attB, att2B], axis=1).astype(bfdt)
    alpha = np.full((P, 1), NEG_SLOPE, np.float32)
    colidx = np.arange(P, dtype=np.float32)[:, None]
    constf = np.concatenate([alpha, colidx], axis=1).astype(np.float32)
    ones1 = np.ones((1, P), bfdt)

    # weights: w1lr[p, k*512 + c] = concat(W1l, W1r)[k*128+p, c]
    W1cat = np.concatenate([W1l, W1r], axis=1)          # [DIN, 2*HC]
    w1lr = np.ascontiguousarray(
        W1cat.reshape(KD, P, 2 * HC).transpose(1, 0, 2).reshape(P, KD * 2 * HC)
    ).astype(bfdt)
    W2cat = np.concatenate([W2l, W2r], axis=1)          # [HC, 2*CO]
    w2lr = np.ascontiguousarray(
        W2cat.reshape(KH, P, 2 * CO).transpose(1, 0, 2).reshape(P, KH * 2 * CO)
    ).astype(bfdt)

    in_maps = []
    for c in range(NC):
        xkc = np.zeros((NPC, DIN), np.float32)
        sel = pp["node_order"][c]
        real = sel >= 0
        xkc[real] = x[sel[real]]
        # xkT[p, (nt*KD+k)*128 + j] = xkc[nt*128 + j, k*128 + p]
        xkT = np.ascontiguousarray(
            xkc.reshape(NT, P, KD, P).transpose(3, 0, 2, 1).reshape(P, NT * KD * P)
        ).astype(bfdt)
        in_maps.append(dict(
            xkT=xkT, w1lr=w1lr, w2lr=w2lr, consts=consts, constf=constf,
            ones1=ones1,
            gidx=np.ascontiguousarray(pp["gidx"][c]),
            drelD=np.ascontiguousarray(
                pp["drelD"][c].reshape(P, -1)).astype(bfdt),
            drelT=np.ascontiguousarray(pp["drelT"][c]).astype(bfdt),
        ))

    from concourse.bass_utils import run_bass_kernel_spmd
    res = run_bass_kernel_spmd(nc, in_maps, core_ids=list(range(NC)))

    h = np.empty((N, CO), np.float32)
    ls = np.empty((N, CO), np.float32)
    r_core = pp["core_of"]
    r_loc = pp["local_of"]
    for c in range(NC):
        m = r_core == c
        h[m] = res.results[c]["h2o"][r_loc[m]]
        ls[m] = res.results[c]["lso"][r_loc[m]]
    return h, ls


# revision 8
# speedup vs baseline: 1.0244x; 1.0244x over previous
"""2-layer GATv2 (PyG GATv2Conv semantics) on 8 Trainium2 NeuronCores — v2.

Improvements over v1:
  - bf16 matmul/table/gather datapath (fp32 PSUM accumulation + epilogues).
  - Self-loop edges are NOT gathered: their message/score path is computed
    batched from the resident per-core xl/xr tiles (overlapping the
    AllGather) and injected into the scatter accumulator with one
    identity-stationary matmul per node tile.  This also kills the lo/hi
    table-half imbalance (self edges all hit the own half) and all the
    fake-pad-edge machinery.
  - Host pre-transposes x, so layer-1 projections need no PE transposes;
    W1l|W1r are concatenated into one 512-wide moving operand.
  - One-hot matrices: oh_en (edge-major) via one batched DVE compare per
    half, oh_s (dst-major) via a rank-1 PE broadcast of the host-provided
    transposed slot row + one batched compare.  No per-tile PE transpose,
    no per-tile PSUM->SBUF copies.
  - All per-edge-tile DVE/ACT elementwise ops are batched per (node tile,
    table half): one Prelu, one att-mult, one segmented reduce, one Exp,
    one msg-mult.  Broadcast operands use pair-duplicated layouts to keep
    the DVE in 2x packed mode.
  - Layer 2: ex folded into the one-hot (ohx), denominator via a
    1.0-column in the gather table rows, log_softmax batched across all
    node tiles.
"""

import sys
import os

if "/opt/trn_rl_repo" not in sys.path:
    sys.path.insert(0, "/opt/trn_rl_repo")

import numpy as np
import ml_dtypes

NC = 8          # cores
P = 128         # partitions
NEG_SLOPE = 0.2

_plan_cache = {}


# --------------------------------------------------------------------------
# host-side graph preprocessing
# --------------------------------------------------------------------------

def _snake(order, nbins):
    n = len(order)
    ids = np.arange(n)
    round_ = ids // nbins
    pos = ids % nbins
    b = np.where(round_ % 2 == 0, pos, nbins - 1 - pos)
    out = np.empty(n, np.int64)
    out[:] = b
    return out


def _preprocess(N, E, edge_index):
    NPC = ((N + NC - 1) // NC + P - 1) // P * P    # padded nodes per core
    NT = NPC // P
    TBL = NC * NPC
    assert TBL // 2 < 32768, "table half must fit int16 row indices"

    src = edge_index[0].astype(np.int64)           # real edges only
    dst = edge_index[1].astype(np.int64)
    deg = np.bincount(dst, minlength=N)

    # --- core assignment: snake over degree-sorted nodes
    order = np.argsort(-deg, kind="stable")
    core_of = np.empty(N, np.int64)
    core_of[order] = _snake(order, NC)

    lo_src = core_of[src] < NC // 2                # which table half each edge reads
    deg_lo = np.bincount(dst[lo_src], minlength=N)
    deg_hi = deg - deg_lo

    # --- per-core tile packing: greedy 2D balance of (lo, hi) in-edges
    local_of = np.empty(N, np.int64)
    for c in range(NC):
        nodes = np.where(core_of == c)[0]
        nodes = nodes[np.argsort(-deg[nodes], kind="stable")]
        Lt = np.zeros(NT)
        Ht = np.zeros(NT)
        cnt = np.zeros(NT, np.int64)
        lo_v = deg_lo[nodes].astype(np.float64)
        hi_v = deg_hi[nodes].astype(np.float64)
        tile_ids = np.empty(len(nodes), np.int64)
        for i in range(len(nodes)):
            cost = np.maximum(Lt + lo_v[i], Ht + hi_v[i])
            cost[cnt >= P] = np.inf
            tl = int(np.argmin(cost))
            tile_ids[i] = tl
            Lt[tl] += lo_v[i]
            Ht[tl] += hi_v[i]
            cnt[tl] += 1
        slot = np.zeros(NT, np.int64)
        for v, tl in zip(nodes, tile_ids):
            local_of[v] = tl * P + slot[tl]
            slot[tl] += 1

    r_of = core_of * NPC + local_of                # global table row of each node

    # --- per (core, tile) edge lists split by half
    e_core = core_of[dst]
    e_tile = local_of[dst] // P
    e_slot = local_of[dst] % P
    cnt_lo = np.zeros((NC, NT), np.int64)
    cnt_hi = np.zeros((NC, NT), np.int64)
    np.add.at(cnt_lo, (e_core[lo_src], e_tile[lo_src]), 1)
    np.add.at(cnt_hi, (e_core[~lo_src], e_tile[~lo_src]), 1)
    K_lo = int(max(1, ((cnt_lo + P - 1) // P).max()))
    K_hi = int(max(1, ((cnt_hi + P - 1) // P).max()))
    T = K_lo + K_hi

    def pack_idx(flat):
        n = len(flat)
        s = (n + 15) // 16
        arr = np.zeros(s * 16, np.int16)
        arr[:n] = flat
        block = arr.reshape(s, 16).T
        return np.tile(block, (8, 1))

    gidx = np.zeros((NC, P, NT * T * 8), np.int16)
    drel = np.full((NC, P, NT * T), -1.0, np.float32)
    for c in range(NC):
        m_c = e_core == c
        for tl in range(NT):
            m = m_c & (e_tile == tl)
            ml = m & lo_src
            mh = m & ~lo_src
            for half, (K, rows0, slots0) in enumerate(
                    [(K_lo, r_of[src[ml]], e_slot[ml]),
                     (K_hi, r_of[src[mh]] - TBL // 2, e_slot[mh])]):
                n = len(rows0)
                flat = np.zeros(K * P, np.int64)
                flat[:n] = rows0
                off = (tl * T + (K_lo if half else 0)) * 8
                gidx[c, :, off:off + K * 8] = pack_idx(flat)
                dr = np.full(K * P, -1.0, np.float32)
                dr[:n] = slots0
                tcol0 = tl * T + (K_lo if half else 0)
                drel[c, :, tcol0:tcol0 + K] = dr.reshape(K, P).T

    # pair-duplicated drel (for the oh_en compare broadcast) and the
    # transposed flat slot row (for the rank-1 oh_s broadcast)
    drelD = np.repeat(drel, 2, axis=2).reshape(NC, P, NT * T, 2)
    drelT = np.ascontiguousarray(
        np.transpose(drel, (0, 2, 1)).reshape(NC, 1, NT * T * P))

    node_order = np.full((NC, NPC), -1, np.int64)  # local row -> global node id
    for c in range(NC):
        nodes = np.where(core_of == c)[0]
        node_order[c, local_of[nodes]] = nodes

    return dict(NPC=NPC, NT=NT, TBL=TBL, K_lo=K_lo, K_hi=K_hi, T=T,
                gidx=gidx, drel=drel, drelD=drelD, drelT=drelT,
                node_order=node_order, core_of=core_of, local_of=local_of)


# --------------------------------------------------------------------------
# bass program
# --------------------------------------------------------------------------

def _build_program(dims, post_passes=True):
    import concourse.bass as bass
    import concourse.mybir as mybir
    import concourse.tile as tile
    from concourse import library_config
    from concourse.bass import _add_dep_helper
    import bass_rust as _br

    fp32 = mybir.dt.float32
    bf = mybir.dt.bfloat16
    i16 = mybir.dt.int16
    AX = mybir.AxisListType
    OP = mybir.AluOpType
    AF = mybir.ActivationFunctionType

    DIN = dims["DIN"]; HC = dims["HC"]; H = dims["H"]; CH = dims["CH"]
    CO = dims["CO"]
    NPC = dims["NPC"]; NT = dims["NT"]; TBL = dims["TBL"]
    K_lo = dims["K_lo"]; K_hi = dims["K_hi"]; T = dims["T"]
    KD = DIN // P
    KH = HC // P
    CO_PAD = 128                       # 256B gather rows in bf16
    HALF = TBL // 2
    OHS_ACT = os.environ.get("GAT_OHS_ACT", "1") == "1"
    SHARED = os.environ.get("GAT_SHARED", "0") == "1"

    nc = bass.Bass(num_devices=NC)

    xkT = nc.dram_tensor("xkT", [P, NT * KD * P], bf, kind="ExternalInput")
    w1lr = nc.dram_tensor("w1lr", [P, KD * 2 * HC], bf, kind="ExternalInput")
    w2lr = nc.dram_tensor("w2lr", [P, KH * 2 * CO], bf, kind="ExternalInput")
    # consts: iota(P) | ident(P) | attB(HC) | att2B(CO)
    CC = P + P + HC + CO
    consts = nc.dram_tensor("consts", [P, CC], bf, kind="ExternalInput")
    constf = nc.dram_tensor("constf", [P, 2], fp32, kind="ExternalInput")  # alpha|colidx
    ones1 = nc.dram_tensor("ones1", [1, P], bf, kind="ExternalInput")
    gidx_d = nc.dram_tensor("gidx", [P, NT * T * 8], i16, kind="ExternalInput")
    drelD_d = nc.dram_tensor("drelD", [P, NT * T * 2], bf, kind="ExternalInput")
    drelT_d = nc.dram_tensor("drelT", [1, NT * T * P], bf, kind="ExternalInput")
    h2_out = nc.dram_tensor("h2o", [NPC, CO], fp32, kind="ExternalOutput")
    ls_out = nc.dram_tensor("lso", [NPC, CO], fp32, kind="ExternalOutput")
    DEBUG = os.environ.get("GAT_DEBUG", "0") == "1"
    if DEBUG:
        xl1_o = nc.dram_tensor("xl1o", [NPC, HC], bf, kind="ExternalOutput")
        xr1_o = nc.dram_tensor("xr1o", [NPC, HC], bf, kind="ExternalOutput")
        m1_o = nc.dram_tensor("m1o", [NPC, HC + H], bf, kind="ExternalOutput")
        h1_o = nc.dram_tensor("h1o", [NPC, HC], bf, kind="ExternalOutput")
        x2_o = nc.dram_tensor("x2o", [NPC, 2 * CO], bf, kind="ExternalOutput")
        glo_o = nc.dram_tensor("gloo", [P, K_lo * HC], bf, kind="ExternalOutput")
        ohe_o = nc.dram_tensor("oheo", [P, K_lo * P], bf, kind="ExternalOutput")
        ohs_o = nc.dram_tensor("ohso", [P, K_lo * P], bf, kind="ExternalOutput")
        md_o = nc.dram_tensor("mdo", [P, K_lo * (HC + H)], bf,
                              kind="ExternalOutput")
        acc_o = nc.dram_tensor("acco", [P, HC + H], fp32, kind="ExternalOutput")

    shr = dict(addr_space="Shared") if SHARED else {}

    with tile.TileContext(nc) as tc:
        with (
            tc.tile_pool(name="dram", bufs=1, space="DRAM") as dram,
            tc.tile_pool(name="cst", bufs=1) as cst,
        ):
            lib = nc.gpsimd.load_library(library_config.mlp)
            reg_klo = nc.gpsimd.to_reg(K_lo * P)
            reg_khi = nc.gpsimd.to_reg(K_hi * P)

            ctile = cst.tile([P, CC], bf)
            nc.sync.dma_start(out=ctile[:], in_=consts[:])
            cftile = cst.tile([P, 2], fp32)
            nc.sync.dma_start(out=cftile[:], in_=constf[:])
            iota = ctile[:, 0:P]
            ident = ctile[:, P:2 * P]
            attB = ctile[:, 2 * P:2 * P + HC]
            att2B = ctile[:, 2 * P + HC:2 * P + HC + CO]
            alpha = cftile[:, 0:1]
            colidx = cftile[:, 1:2]

            ones_sb = cst.tile([1, P], bf)
            nc.sync.dma_start(out=ones_sb[:], in_=ones1[:])

            w1lr_sb = cst.tile([P, KD, 2 * HC], bf)
            nc.sync.dma_start(out=w1lr_sb[:],
                              in_=w1lr.rearrange("p (k c) -> p k c", k=KD))
            w2lr_sb = cst.tile([P, KH, 2 * CO], bf)
            nc.sync.dma_start(out=w2lr_sb[:],
                              in_=w2lr.rearrange("p (k c) -> p k c", k=KH))

            gidx_sb = cst.tile([P, NT * T * 8], i16)
            nc.sync.dma_start(out=gidx_sb[:], in_=gidx_d[:])
            drelD_sb = cst.tile([P, NT * T, 2], bf)
            nc.sync.dma_start(out=drelD_sb[:],
                              in_=drelD_d.rearrange("p (t two) -> p t two", two=2))

            xl1_all = cst.tile([P, NT, HC], bf)
            xr1_all = cst.tile([P, NT, HC], bf)
            msg1s = cst.tile([P, NT, HC + H], bf)
            h1_all = cst.tile([P, NT, HC], bf)
            xl2_all = cst.tile([P, NT, CO], bf)
            xr2_all = cst.tile([P, NT, CO], bf)
            msg2s = cst.tile([P, NT, CO + 1], bf)
            h2_all = cst.tile([P, NT, CO], fp32)
            ls_all = cst.tile([P, NT, CO], fp32)

            ag1_in = dram.tile([NPC, HC], bf)
            tbl1 = dram.tile([TBL, HC], bf, **shr)
            ag2_in = dram.tile([NPC, CO_PAD], bf)
            tbl2 = dram.tile([TBL, CO_PAD], bf, **shr)

            # ============ phase A: layer-1 projections ============
            with (tc.tile_pool(name="sbA", bufs=2) as sb,
                  tc.tile_pool(name="psA", bufs=2, space="PSUM") as ps):
                for nt in range(NT):
                    xT = sb.tile([P, KD, P], bf, tag="xT")
                    nc.sync.dma_start(
                        out=xT[:],
                        in_=xkT.rearrange("p (n k j) -> p n k j", n=NT, k=KD)[:, nt])
                    lr_ps = ps.tile([P, 2 * HC], fp32, tag="mmA", space="PSUM")
                    for k in range(KD):
                        nc.tensor.matmul(out=lr_ps[:], lhsT=xT[:, k, :],
                                         rhs=w1lr_sb[:, k, :],
                                         start=(k == 0), stop=(k == KD - 1))
                    nc.vector.tensor_copy(out=xl1_all[:, nt, :], in_=lr_ps[:, 0:HC])
                    nc.vector.tensor_copy(out=xr1_all[:, nt, :],
                                          in_=lr_ps[:, HC:2 * HC])
                    nc.sync.dma_start(out=ag1_in[nt * P:(nt + 1) * P, :],
                                      in_=xl1_all[:, nt, :])

            nc.gpsimd.collective_compute(
                "AllGather", mybir.AluOpType.bypass,
                replica_groups=[list(range(NC))],
                ins=[ag1_in[:].opt()], outs=[tbl1[:].opt()],
            )

            # ---- self-edge pass (overlaps AG1: no dependence on tbl1) ----
            with tc.tile_pool(name="sbS", bufs=2) as sb:
                CH8 = 8
                for c0 in range(0, NT, CH8):
                    n = min(CH8, NT - c0)
                    sl = slice(c0, c0 + n)
                    zs = sb.tile([P, CH8, HC], bf, tag="zs")
                    nc.vector.tensor_tensor(out=zs[:, 0:n], in0=xl1_all[:, sl],
                                            in1=xr1_all[:, sl], op=OP.add)
                    ts = sb.tile([P, CH8, HC], bf, tag="ts")
                    nc.scalar.activation(out=ts[:, 0:n], in_=zs[:, 0:n],
                                         func=AF.Prelu, alpha=alpha)
                    nc.vector.tensor_tensor(
                        out=ts[:, 0:n], in0=ts[:, 0:n],
                        in1=attB[:, None, :].to_broadcast([P, n, HC]), op=OP.mult)
                    scs = sb.tile([P, CH8, H], fp32, tag="scs")
                    nc.vector.tensor_reduce(
                        out=scs[:, 0:n],
                        in_=ts[:, 0:n].rearrange("p n (h c) -> p n h c", h=H),
                        axis=AX.X, op=OP.add)
                    nc.scalar.activation(out=msg1s[:, sl, HC:HC + H],
                                         in_=scs[:, 0:n], func=AF.Exp)
                    exd = sb.tile([P, CH8, H, 2], bf, tag="exd")
                    nc.vector.tensor_copy(
                        out=exd[:, 0:n],
                        in_=msg1s[:, sl, HC:HC + H][:, :, :, None]
                        .to_broadcast([P, n, H, 2]))
                    nc.vector.tensor_tensor(
                        out=msg1s[:, sl, 0:HC].rearrange(
                            "p n (h c two) -> p n h c two", h=H, two=2),
                        in0=xl1_all[:, sl].rearrange(
                            "p n (h c two) -> p n h c two", h=H, two=2),
                        in1=exd[:, 0:n, :, None, :].to_broadcast(
                            [P, n, H, CH // 2, 2]),
                        op=OP.mult)

            # ============ phase B: layer-1 edges ============
            with (tc.tile_pool(name="sbB", bufs=2) as sb,
                  tc.tile_pool(name="psB", bufs=1, space="PSUM") as ps,
                  tc.tile_pool(name="psBa", bufs=2, space="PSUM") as psa):
                for nt in range(NT):
                    drelT_sb = sb.tile([1, T * P], bf, tag="drelT")
                    nc.sync.dma_start(out=drelT_sb[:],
                                      in_=drelT_d[:, nt * T * P:(nt + 1) * T * P])
                    glo = sb.tile([P, K_lo, HC], bf, tag="glo")
                    ghi = sb.tile([P, K_hi, HC], bf, tag="ghi")
                    off = nt * T * 8
                    g1 = nc.gpsimd.dma_gather(
                        glo[:], tbl1[0:HALF, :], gidx_sb[:, off:off + K_lo * 8],
                        K_lo * P, reg_klo, HC)
                    g2 = nc.gpsimd.dma_gather(
                        ghi[:], tbl1[HALF:TBL, :],
                        gidx_sb[:, off + K_lo * 8:off + T * 8],
                        K_hi * P, reg_khi, HC)
                    _add_dep_helper(g1.ins, lib.ins, sync=False, reason="lib")
                    _add_dep_helper(g2.ins, lib.ins, sync=False, reason="lib")

                    acc = psa.tile([P, HC + H], fp32, tag="acc", space="PSUM")
                    nc.tensor.matmul(out=acc[:], lhsT=ident, rhs=msg1s[:, nt, :],
                                     start=True, stop=False)

                    for half in range(2):
                        K = K_lo if half == 0 else K_hi
                        g = glo if half == 0 else ghi
                        t0c = nt * T + (K_lo if half else 0)
                        doff = (K_lo if half else 0) * P
                        z = ps.tile([P, K, HC], fp32, tag="z", space="PSUM")
                        zf = z[:].rearrange("p k c -> p (k c)")
                        ohe = sb.tile([P, K, P], bf, tag=f"ohe{half}")
                        nc.vector.tensor_tensor(
                            out=ohe[:].rearrange("p k (q two) -> p k q two", two=2),
                            in0=iota[:, None, :].to_broadcast([P, K, P]).rearrange(
                                "p k (q two) -> p k q two", two=2),
                            in1=drelD_sb[:, t0c:t0c + K, None, :].to_broadcast(
                                [P, K, P // 2, 2]),
                            op=OP.is_equal)
                        # oh_s = oh_en^T via PE transpose (dedicated PSUM area)
                        ohsT = ps.tile([P, K_lo, P], fp32, tag="ohsT",
                                       space="PSUM")
                        for t in range(K):
                            nc.tensor.matmul(out=ohsT[:, t, :],
                                             lhsT=ohe[:, t, :], rhs=ident,
                                             start=True, stop=True)
                        ohs = sb.tile([P, K, P], bf, tag=f"ohs{half}")
                        nc.scalar.copy(
                            out=ohs[:].rearrange("p k q -> p (k q)"),
                            in_=ohsT[:].rearrange("p k q -> p (k q)")[:, 0:K * P])
                        # z = xr[dst] + xl[src], per-tile paired accumulation
                        for t in range(K):
                            nc.tensor.matmul(out=z[:, t, :], lhsT=ohs[:, t, :],
                                             rhs=xr1_all[:, nt, :],
                                             start=True, stop=False)
                            nc.tensor.matmul(out=z[:, t, :], lhsT=ident,
                                             rhs=g[:, t, :],
                                             start=False, stop=True)
                        tb = sb.tile([P, K, HC], bf, tag=f"tb{half}")
                        nc.scalar.activation(
                            out=tb[:].rearrange("p k c -> p (k c)"),
                            in_=zf[:], func=AF.Prelu, alpha=alpha)
                        nc.vector.tensor_tensor(
                            out=tb[:], in0=tb[:],
                            in1=attB[:, None, :].to_broadcast([P, K, HC]),
                            op=OP.mult)
                        scf = sb.tile([P, K, H], fp32, tag=f"sc{half}")
                        nc.vector.tensor_reduce(
                            out=scf[:],
                            in_=tb[:].rearrange("p k (h c) -> p k h c", h=H),
                            axis=AX.X, op=OP.add)
                        md = sb.tile([P, K, HC + H], bf, tag=f"md{half}")
                        nc.scalar.activation(out=md[:, :, HC:HC + H], in_=scf[:],
                                             func=AF.Exp)
                        exd = sb.tile([P, K, H, 2], bf, tag=f"exd{half}")
                        nc.scalar.activation(out=exd[:, :, :, 0], in_=scf[:],
                                             func=AF.Exp)
                        nc.scalar.activation(out=exd[:, :, :, 1], in_=scf[:],
                                             func=AF.Exp)
                        nc.vector.tensor_tensor(
                            out=md[:, :, 0:HC].rearrange(
                                "p k (h c two) -> p k h c two", h=H, two=2),
                            in0=g[:].rearrange(
                                "p k (h c two) -> p k h c two", h=H, two=2),
                            in1=exd[:, :, :, None, :].to_broadcast(
                                [P, K, H, CH // 2, 2]),
                            op=OP.mult)
                        for t in range(K):
                            nc.tensor.matmul(
                                out=acc[:], lhsT=ohe[:, t, :], rhs=md[:, t, :],
                                start=False, stop=(half == 1 and t == K - 1))
                        if DEBUG and nt == 0 and half == 0:
                            nc.sync.dma_start(
                                out=glo_o[:],
                                in_=g[:].rearrange("p k c -> p (k c)"))
                            nc.sync.dma_start(
                                out=ohe_o[:],
                                in_=ohe[:].rearrange("p k q -> p (k q)"))
                            nc.sync.dma_start(
                                out=ohs_o[:],
                                in_=ohs[:].rearrange("p k q -> p (k q)"))
                            nc.sync.dma_start(
                                out=md_o[:],
                                in_=md[:].rearrange("p k c -> p (k c)"))

                    if DEBUG and nt == 0:
                        accc = sb.tile([P, HC + H], fp32, tag="accc")
                        nc.vector.tensor_copy(out=accc[:], in_=acc[:])
                        nc.sync.dma_start(out=acc_o[:], in_=accc[:])

                    # ---- per-node epilogue: softmax divide, elu, layer-2 proj
                    rec = sb.tile([P, H], fp32, tag="rec")
                    nc.vector.reciprocal(out=rec[:], in_=acc[:, HC:HC + H])
                    recd = sb.tile([P, H, 2], fp32, tag="recd")
                    nc.vector.tensor_copy(
                        out=recd[:], in_=rec[:, :, None].to_broadcast([P, H, 2]))
                    nc.vector.tensor_tensor(
                        out=h1_all[:, nt, :].rearrange(
                            "p (h c two) -> p h c two", h=H, two=2),
                        in0=acc[:, 0:HC].rearrange(
                            "p (h c two) -> p h c two", h=H, two=2),
                        in1=recd[:, :, None, :].to_broadcast([P, H, CH // 2, 2]),
                        op=OP.mult)
                    h1v = h1_all[:, nt, :]
                    ehb = sb.tile([P, HC], bf, tag="ehb")
                    nc.scalar.activation(out=ehb[:], in_=h1v, func=AF.Exp)
                    em = sb.tile([P, HC], bf, tag="em")
                    nc.vector.tensor_scalar(out=em[:], in0=ehb[:], scalar1=1.0,
                                            scalar2=0.0, op0=OP.subtract,
                                            op1=OP.min)
                    elu = sb.tile([P, HC], bf, tag="elu")
                    nc.vector.tensor_scalar(out=elu[:], in0=h1v, scalar1=0.0,
                                            scalar2=None, op0=OP.max)
                    nc.vector.tensor_tensor(out=elu[:], in0=elu[:], in1=em[:],
                                            op=OP.add)
                    epi = ps.tile([P, KH * P + 2 * CO], fp32, tag="epi",
                                  space="PSUM")
                    hT_ps = epi[:, 0:KH * P].rearrange("p (k q) -> p k q", k=KH)
                    for k in range(KH):
                        nc.tensor.matmul(out=hT_ps[:, k, :],
                                         lhsT=elu[:, k * P:(k + 1) * P],
                                         rhs=ident, start=True, stop=True)
                    hT_sb = sb.tile([P, KH, P], bf, tag="hTs")
                    nc.vector.tensor_copy(out=hT_sb[:], in_=hT_ps[:])
                    x2_ps = epi[:, KH * P:KH * P + 2 * CO]
                    for k in range(KH):
                        nc.tensor.matmul(out=x2_ps[:], lhsT=hT_sb[:, k, :],
                                         rhs=w2lr_sb[:, k, :],
                                         start=(k == 0), stop=(k == KH - 1))
                    nc.vector.tensor_copy(out=xl2_all[:, nt, :],
                                          in_=x2_ps[:, 0:CO])
                    nc.vector.tensor_copy(out=xr2_all[:, nt, :],
                                          in_=x2_ps[:, CO:2 * CO])
                    x2c = sb.tile([P, CO + 1], bf, tag="x2c")
                    nc.vector.tensor_copy(out=x2c[:, 0:CO], in_=xl2_all[:, nt, :])
                    nc.vector.memset(x2c[:, CO:CO + 1], 1.0)
                    nc.sync.dma_start(
                        out=ag2_in[nt * P:(nt + 1) * P, 0:CO + 1], in_=x2c[:])

            nc.gpsimd.collective_compute(
                "AllGather", mybir.AluOpType.bypass,
                replica_groups=[list(range(NC))],
                ins=[ag2_in[:].opt()], outs=[tbl2[:].opt()],
            )

            # ---- layer-2 self-edge pass (overlaps AG2) ----
            with tc.tile_pool(name="sbS2", bufs=1) as sb:
                z2s = sb.tile([P, NT, CO], bf)
                nc.vector.tensor_tensor(out=z2s[:], in0=xl2_all[:],
                                        in1=xr2_all[:], op=OP.add)
                t2s = sb.tile([P, NT, CO], bf)
                nc.scalar.activation(out=t2s[:], in_=z2s[:], func=AF.Prelu,
                                     alpha=alpha)
                nc.vector.tensor_tensor(
                    out=t2s[:], in0=t2s[:],
                    in1=att2B[:, None, :].to_broadcast([P, NT, CO]), op=OP.mult)
                sc2s = sb.tile([P, NT], fp32)
                nc.vector.tensor_reduce(out=sc2s[:], in_=t2s[:], axis=AX.X,
                                        op=OP.add)
                ex2s = sb.tile([P, NT], bf)
                nc.scalar.activation(out=ex2s[:], in_=sc2s[:], func=AF.Exp)
                nc.vector.tensor_copy(out=msg2s[:, :, CO:CO + 1],
                                      in_=ex2s[:, :, None])
                ex2sd = sb.tile([P, NT, 2], bf)
                nc.vector.tensor_copy(
                    out=ex2sd[:], in_=ex2s[:, :, None].to_broadcast([P, NT, 2]))
                nc.vector.tensor_tensor(
                    out=msg2s[:, :, 0:CO].rearrange(
                        "p n (c two) -> p n c two", two=2),
                    in0=xl2_all[:].rearrange("p n (c two) -> p n c two", two=2),
                    in1=ex2sd[:, :, None, :].to_broadcast([P, NT, CO // 2, 2]),
                    op=OP.mult)

            # ============ phase C: layer-2 edges ============
            with (tc.tile_pool(name="sbC", bufs=2) as sb,
                  tc.tile_pool(name="psC", bufs=1, space="PSUM") as ps,
                  tc.tile_pool(name="psCa", bufs=2, space="PSUM") as psa):
                for nt in range(NT):
                    drelT_sb = sb.tile([1, T * P], bf, tag="drelT")
                    nc.sync.dma_start(out=drelT_sb[:],
                                      in_=drelT_d[:, nt * T * P:(nt + 1) * T * P])
                    g2lo = sb.tile([P, K_lo, CO_PAD], bf, tag="g2lo")
                    g2hi = sb.tile([P, K_hi, CO_PAD], bf, tag="g2hi")
                    off = nt * T * 8
                    g1 = nc.gpsimd.dma_gather(
                        g2lo[:], tbl2[0:HALF, :], gidx_sb[:, off:off + K_lo * 8],
                        K_lo * P, reg_klo, CO_PAD)
                    g2 = nc.gpsimd.dma_gather(
                        g2hi[:], tbl2[HALF:TBL, :],
                        gidx_sb[:, off + K_lo * 8:off + T * 8],
                        K_hi * P, reg_khi, CO_PAD)
                    _add_dep_helper(g1.ins, lib.ins, sync=False, reason="lib")
                    _add_dep_helper(g2.ins, lib.ins, sync=False, reason="lib")

                    acc2 = psa.tile([P, CO + 1], fp32, tag="acc2", space="PSUM")
                    nc.tensor.matmul(out=acc2[:], lhsT=ident, rhs=msg2s[:, nt, :],
                                     start=True, stop=False)

                    z2 = ps.tile([P, T, CO], fp32, tag="z2", space="PSUM")
                    z2b = sb.tile([P, T, CO], bf, tag="z2b")
                    t2 = sb.tile([P, T, CO], bf, tag="t2")
                    sc2 = sb.tile([P, T], fp32, tag="sc2")
                    ex2 = sb.tile([P, T], bf, tag="ex2")
                    exd2 = sb.tile([P, T, 2], bf, tag="exd2")
                    ohs_l = []
                    ohe_l = []
                    for half in range(2):
                        K = K_lo if half == 0 else K_hi
                        tg = 0 if half == 0 else K_lo
                        t0c = nt * T + tg
                        doff = tg * P
                        dT = ps.tile([P, K_lo * P], fp32, tag=f"dT{half}",
                                     space="PSUM")
                        ohe = sb.tile([P, K, P], bf, tag=f"ohe{half}")
                        nc.vector.tensor_tensor(
                            out=ohe[:].rearrange("p k (q two) -> p k q two", two=2),
                            in0=iota[:, None, :].to_broadcast([P, K, P]).rearrange(
                                "p k (q two) -> p k q two", two=2),
                            in1=drelD_sb[:, t0c:t0c + K, None, :].to_broadcast(
                                [P, K, P // 2, 2]),
                            op=OP.is_equal)
                        dTv = dT[:, 0:K * P].rearrange("p (k q) -> p k q", k=K)
                        for t in range(K):
                            nc.tensor.matmul(out=dTv[:, t, :],
                                             lhsT=ohe[:, t, :], rhs=ident,
                                             start=True, stop=True)
                        ohs = sb.tile([P, K, P], bf, tag=f"ohs{half}")
                        nc.scalar.copy(out=ohs[:].rearrange("p k q -> p (k q)"),
                                       in_=dT[:, 0:K * P])
                        ohs_l.append(ohs)
                        ohe_l.append(ohe)
                        for t in range(K):
                            nc.tensor.matmul(out=z2[:, tg + t, :],
                                             lhsT=ohs[:, t, :],
                                             rhs=xr2_all[:, nt, :],
                                             start=True, stop=True)
                        g = g2lo if half == 0 else g2hi
                        nc.vector.tensor_tensor(
                            out=z2b[:, tg:tg + K, :], in0=z2[:, tg:tg + K, :],
                            in1=g[:, :, 0:CO], op=OP.add)
                    nc.scalar.activation(
                        out=t2[:].rearrange("p t c -> p (t c)"),
                        in_=z2b[:].rearrange("p t c -> p (t c)"),
                        func=AF.Prelu, alpha=alpha)
                    nc.vector.tensor_tensor(
                        out=t2[:], in0=t2[:],
                        in1=att2B[:, None, :].to_broadcast([P, T, CO]),
                        op=OP.mult)
                    nc.vector.tensor_reduce(out=sc2[:], in_=t2[:], axis=AX.X,
                                            op=OP.add)
                    nc.scalar.activation(out=ex2[:], in_=sc2[:], func=AF.Exp)
                    nc.vector.tensor_copy(
                        out=exd2[:], in_=ex2[:, :, None].to_broadcast([P, T, 2]))
                    for half in range(2):
                        K = K_lo if half == 0 else K_hi
                        tg = 0 if half == 0 else K_lo
                        g = g2lo if half == 0 else g2hi
                        ohx = sb.tile([P, K, P], bf, tag=f"ohx{half}")
                        nc.vector.tensor_tensor(
                            out=ohx[:].rearrange("p k (q two) -> p k q two", two=2),
                            in0=ohe_l[half][:].rearrange(
                                "p k (q two) -> p k q two", two=2),
                            in1=exd2[:, tg:tg + K, None, :].to_broadcast(
                                [P, K, P // 2, 2]),
                            op=OP.mult)
                        for t in range(K):
                            nc.tensor.matmul(
                                out=acc2[:], lhsT=ohx[:, t, :],
                                rhs=g[:, t, 0:CO + 1],
                                start=False, stop=(half == 1 and t == K - 1))
                    rec2 = sb.tile([P, 1], fp32, tag="rec2")
                    nc.vector.reciprocal(out=rec2[:], in_=acc2[:, CO:CO + 1])
                    nc.vector.tensor_scalar(out=h2_all[:, nt, :],
                                            in0=acc2[:, 0:CO],
                                            scalar1=rec2[:, 0:1], scalar2=None,
                                            op0=OP.mult)

            # ============ phase D: batched log_softmax ============
            with tc.tile_pool(name="sbD", bufs=1) as sb:
                nm = sb.tile([P, NT], fp32)
                nc.vector.tensor_reduce(out=nm[:], in_=h2_all[:], axis=AX.X,
                                        op=OP.max, negate=True)
                hs = sb.tile([P, NT, CO], fp32)
                nc.vector.tensor_tensor(
                    out=hs[:], in0=h2_all[:],
                    in1=nm[:, :, None].to_broadcast([P, NT, CO]), op=OP.add)
                es = sb.tile([P, NT, CO], fp32)
                nc.scalar.activation(out=es[:].rearrange("p t c -> p (t c)"),
                                     in_=hs[:].rearrange("p t c -> p (t c)"),
                                     func=AF.Exp)
                ssum = sb.tile([P, NT], fp32)
                nc.vector.tensor_reduce(out=ssum[:], in_=es[:], axis=AX.X,
                                        op=OP.add)
                lns = sb.tile([P, NT], fp32)
                nc.scalar.activation(out=lns[:], in_=ssum[:], func=AF.Ln)
                nc.vector.tensor_tensor(
                    out=ls_all[:], in0=hs[:],
                    in1=lns[:, :, None].to_broadcast([P, NT, CO]),
                    op=OP.subtract)

            nc.sync.dma_start(out=h2_out.rearrange("(a p) d -> p a d", p=P),
                              in_=h2_all[:])
            nc.sync.dma_start(out=ls_out.rearrange("(a p) d -> p a d", p=P),
                              in_=ls_all[:])
            if DEBUG:
                for name_t, src_t in [(xl1_o, xl1_all), (xr1_o, xr1_all),
                                      (m1_o, msg1s), (h1_o, h1_all)]:
                    nc.sync.dma_start(
                        out=name_t.rearrange("(a p) d -> p a d", p=P),
                        in_=src_t[:])
                nc.sync.dma_start(
                    out=x2_o.rearrange("(a p) (d) -> p a d", p=P)[:, :, 0:CO],
                    in_=xl2_all[:])
                nc.sync.dma_start(
                    out=x2_o.rearrange("(a p) (d) -> p a d", p=P)[:, :, CO:2 * CO],
                    in_=xr2_all[:])

    if post_passes:
        _br.generate_event_semaphores(nc)
        _br.codegen_inst_isa_subclasses(nc)
    return nc


# --------------------------------------------------------------------------
# entry point
# --------------------------------------------------------------------------

def kernel(x, edge_index, W1l, W1r, att1, b1, W2l, W2r, att2, b2):
    x = np.asarray(x, np.float32)
    edge_index = np.asarray(edge_index)
    W1l = np.asarray(W1l, np.float32); W1r = np.asarray(W1r, np.float32)
    att1 = np.asarray(att1, np.float32); b1 = np.asarray(b1, np.float32)
    W2l = np.asarray(W2l, np.float32); W2r = np.asarray(W2r, np.float32)
    att2 = np.asarray(att2, np.float32); b2 = np.asarray(b2, np.float32)

    N, DIN = x.shape
    E = edge_index.shape[1]
    H, CH = att1.shape
    HC = W1l.shape[1]
    CO = W2l.shape[1]
    assert np.abs(b1).max() == 0 and np.abs(b2).max() == 0, "bias not supported"

    key = (N, E, DIN, H, CH, HC, CO, hash(edge_index.tobytes()))
    if key in _plan_cache:
        pp, nc, dims = _plan_cache[key]
    else:
        pp = _preprocess(N, E, edge_index)
        dims = dict(DIN=DIN, HC=HC, H=H, CH=CH, CO=CO,
                    NPC=pp["NPC"], NT=pp["NT"], TBL=pp["TBL"],
                    K_lo=pp["K_lo"], K_hi=pp["K_hi"], T=pp["T"])
        nc = _build_program(dims)
        _plan_cache[key] = (pp, nc, dims)

    NPC = pp["NPC"]; NT = pp["NT"]
    KD = DIN // P
    KH = HC // P
    bfdt = ml_dtypes.bfloat16

    iota = np.broadcast_to(np.arange(P, dtype=np.float32)[None, :], (P, P))
    ident = np.eye(P, dtype=np.float32)
    attB = np.broadcast_to(att1.reshape(1, HC), (P, HC))
    att2B = np.broadcast_to(att2.reshape(1, CO), (P, CO))
    consts = np.concatenate([iota, ident, attB, att2B], axis=1).astype(bfdt)
    alpha = np.full((P, 1), NEG_SLOPE, np.float32)
    colidx = np.arange(P, dtype=np.float32)[:, None]
    constf = np.concatenate([alpha, colidx], axis=1).astype(np.float32)
    ones1 = np.ones((1, P), bfdt)

    # weights: w1lr[p, k*512 + c] = concat(W1l, W1r)[k*128+p, c]
    W1cat = np.concatenate([W1l, W1r], axis=1)          # [DIN, 2*HC]
    w1lr = np.ascontiguousarray(
        W1cat.reshape(KD, P, 2 * HC).transpose(1, 0, 2).reshape(P, KD * 2 * HC)
    ).astype(bfdt)
    W2cat = np.concatenate([W2l, W2r], axis=1)          # [HC, 2*CO]
    w2lr = np.ascontiguousarray(
        W2cat.reshape(KH, P, 2 * CO).transpose(1, 0, 2).reshape(P, KH * 2 * CO)
    ).astype(bfdt)

    in_maps = []
    for c in range(NC):
        xkc = np.zeros((NPC, DIN), np.float32)
        sel = pp["node_order"][c]
        real = sel >= 0
        xkc[real] = x[sel[real]]
        # xkT[p, (nt*KD+k)*128 + j] = xkc[nt*128 + j, k*128 + p]
        xkT = np.ascontiguousarray(
            xkc.reshape(NT, P, KD, P).transpose(3, 0, 2, 1).reshape(P, NT * KD * P)
        ).astype(bfdt)
        in_maps.append(dict(
            xkT=xkT, w1lr=w1lr, w2lr=w2lr, consts=consts, constf=constf,
            ones1=ones1,
            gidx=np.ascontiguousarray(pp["gidx"][c]),
            drelD=np.ascontiguousarray(
                pp["drelD"][c].reshape(P, -1)).astype(bfdt),
            drelT=np.ascontiguousarray(pp["drelT"][c]).astype(bfdt),
        ))

    from concourse.bass_utils import run_bass_kernel_spmd
    res = run_bass_kernel_spmd(nc, in_maps, core_ids=list(range(NC)))

    h = np.empty((N, CO), np.float32)
    ls = np.empty((N, CO), np.float32)
    r_core = pp["core_of"]
    r_loc = pp["local_of"]
    for c in range(NC):
        m = r_core == c
        h[m] = res.results[c]["h2o"][r_loc[m]]
        ls[m] = res.results[c]["lso"][r_loc[m]]
    return h, ls
